# revision 9
# baseline (speedup 1.0000x reference)
"""Multi-head attention layer on 8 TRN2 NeuronCores.

Problem: B=4, S=2048, D=1024, H=16 heads (HD=64). Returns (out, attention)
matching the reference nn.Module: QKV projections, scaled-dot-product
softmax attention (attention probs are a graded output), output projection.

Sharding: core i handles batch b = i//2 and head-group g = i%2 (8 heads).
Weights are column-split (Wq/Wk/Wv) / row-split (Wo) per head-group; each
core computes its [1, 8, S, S] attention slab and a partial output
projection. Host sums the two partial outputs per batch and adds bo.

Per-core pipeline (all layouts chosen so softmax reductions are along the
free dim and DMA writes are row-contiguous):
  1. Transpose inputs X -> X^T via TensorE (fp32r), project to Q^T/K^T
     [dq, S] (head-dim on partitions) and V [S, dv] (fp32r matmuls).
  2. Scores S = Q^T.T @ K^T per 128-row q-block (fp32r, head-dim
     contraction); exp via ScalarE with scale=1/8 and accum_out giving the
     softmax row-sum for free; normalize P = E * (1/Z) on VectorE
     (per-partition scalar, bf16); DMA P to the attention output with
     SWDGE bf16->f32 cast.
  3. P^T via TensorE transpose (bf16); PV matmul accumulates O over
     k-chunks; output projection against row-split Wo (bf16).
"""

import numpy as np

import concourse.bass as bass
import concourse.mybir as mybir
import concourse.tile as tile
from concourse.bass_utils import run_bass_kernel_spmd
from concourse.masks import make_identity
from concourse.vector_clock import ScopedClock

B, S, D, H = 4, 2048, 1024, 16
HD = D // H  # 64
NCORES = 8
HG = H // 2  # heads per core = 8
DG = HG * HD  # 512 per-core projection width

F32 = mybir.dt.float32
F32R = mybir.dt.float32r
BF16 = mybir.dt.bfloat16

# ---------------------------------------------------------------------------
# walrus's setupSyncWait supports only one sem wait on the Tile tail drain;
# split excess waits onto chained nops (same engine => sequential).
_MAX_WAITS = 1


def _split_drain_and_barrier(self, tick_clock, wait_clock):
    nc = self.nc
    drain_inst = nc.sync.drain()
    wait_clock.add_sem_waits(
        drain_inst.ins, ScopedClock({None: tick_clock.global_clock})
    )
    si = drain_inst.ins.sync_info
    if si is not None and si.on_wait and len(si.on_wait) > _MAX_WAITS:
        waits = list(si.on_wait)
        si.on_wait = waits[:_MAX_WAITS]
        rest = waits[_MAX_WAITS:]
        while rest:
            chunk, rest = rest[:_MAX_WAITS], rest[_MAX_WAITS:]
            nop = nc.sync.nop(nofuse=True)
            nop.ins.sync_info = mybir.SyncInfo(on_wait=chunk, on_update=[])

    nc.all_engine_barrier()
    assert self.sems is not None
    popped = nc._tile_sem_poison_stack.pop()
    assert popped is self._sem_poison
    nc.clear_and_free_semaphores(list(self.sems.allocated().values()))
    nc.all_engine_barrier()


tile.TileContext._drain_and_barrier = _split_drain_and_barrier


def _split_excess_waits(nc, max_waits=1):
    """walrus's setupSyncWait rejects instructions with more than one sem
    wait; move extras onto preceding same-engine nops (engine streams are
    in-order, so waits on earlier instructions gate later ones)."""
    uid = [0]
    for f in nc.m.functions:
        for bb in f.blocks:
            new_insts = []
            for inst in bb.instructions:
                si = inst.sync_info
                if si is not None and si.on_wait and len(si.on_wait) > max_waits:
                    waits = list(si.on_wait)
                    si.on_wait = waits[:max_waits]
                    rest = waits[max_waits:]
                    while rest:
                        chunk, rest = rest[:max_waits], rest[max_waits:]
                        uid[0] += 1
                        nop = mybir.InstNoOp(
                            name=f"{inst.name}-w{uid[0]}",
                            sync_info=mybir.SyncInfo(on_wait=chunk, on_update=[]),
                            bass_nofuse=True,
                            engine=inst.engine,
                        )
                        new_insts.append(nop)
                new_insts.append(inst)
            bb.instructions[:] = new_insts


# ---------------------------------------------------------------------------


def build_program():
    nc = bass.Bass()

    xq = nc.declare_dram_parameter("xq", [S, D], F32, isOutput=False)
    xk = nc.declare_dram_parameter("xk", [S, D], F32, isOutput=False)
    xv = nc.declare_dram_parameter("xv", [S, D], F32, isOutput=False)
    wq = nc.declare_dram_parameter("wq", [D, DG], F32R, isOutput=False)
    wk = nc.declare_dram_parameter("wk", [D, DG], F32R, isOutput=False)
    wv = nc.declare_dram_parameter("wv", [D, DG], F32R, isOutput=False)
    wo = nc.declare_dram_parameter("wo", [DG, D], F32, isOutput=False)
    att = nc.declare_dram_parameter("att", [HG, S, S], F32, isOutput=True)
    outp = nc.declare_dram_parameter("outp", [S, D], F32, isOutput=True)

    with tile.TileContext(nc) as tc:
        build_tile_kernel(tc, nc, xq, xk, xv, wq, wk, wv, wo, att, outp)
    _split_excess_waits(nc)
    return nc


def build_tile_kernel(tc, nc, xq, xk, xv, wq, wk, wv, wo, att, outp):
    from contextlib import ExitStack

    with ExitStack() as outer:
        _build_tile_kernel(tc, nc, outer, xq, xk, xv, wq, wk, wv, wo, att, outp)


def _build_tile_kernel(tc, nc, outer, xq, xk, xv, wq, wk, wv, wo, att, outp):
    from contextlib import ExitStack

    consts = outer.enter_context(tc.tile_pool(name="consts", bufs=1))
    ident_f = consts.tile([128, 128], F32)
    make_identity(nc, ident_f)
    ident_b = consts.tile([128, 128], BF16)
    make_identity(nc, ident_b)
    wo_bf = consts.tile([128, 4, D], BF16)
    nc.gpsimd.dma_start(out=wo_bf, in_=wo.rearrange("(pc p) n -> p pc n", p=128))

    resident = outer.enter_context(tc.tile_pool(name="resident", bufs=1))
    qT = resident.tile([128, 4, S], F32R)  # [dq%128, dq-chunk(2 heads), m]
    kT = resident.tile([128, 4, S], F32R)
    v_bf = resident.tile([128, 16, DG], BF16)  # [k%128, k-chunk, dv]
    oT = resident.tile([128, 4, S], BF16)  # [d%128, head-pair, q]

    # ---------------- Phase 1: input transposes + projections --------------
    with ExitStack() as ph1:
        wpool = ph1.enter_context(tc.tile_pool(name="wpool", bufs=1))
        xpool = ph1.enter_context(tc.tile_pool(name="xpool", bufs=5))
        xtpool = ph1.enter_context(tc.tile_pool(name="xtpool", bufs=2))
        ppsum = ph1.enter_context(tc.tile_pool(name="ppsum", bufs=2, space="PSUM"))
        tpsum = ph1.enter_context(tc.tile_pool(name="tpsum", bufs=2, space="PSUM"))

        for x_dram, w_dram, kind in (
            (xq, wq, "q"),
            (xk, wk, "k"),
            (xv, wv, "v"),
        ):
            w_sb = wpool.tile([128, 8, DG], F32R, tag="w")
            nc.sync.dma_start(
                out=w_sb, in_=w_dram.rearrange("(cc p) d -> p cc d", p=128)
            )
            for mb in range(4):  # m-blocks of 512
                x_tiles = []
                for ms in range(4):
                    xt_in = xpool.tile([128, D], F32, tag="x")
                    m0 = mb * 512 + ms * 128
                    nc.sync.dma_start(out=xt_in, in_=x_dram[m0 : m0 + 128, :])
                    x_tiles.append(xt_in)
                xt = xtpool.tile([128, 8, 512], F32R, tag="xt")
                for cc in range(8):
                    tp = tpsum.tile([128, 512], F32, tag="tp")
                    for ms in range(4):
                        nc.tensor.transpose(
                            tp[:, ms * 128 : (ms + 1) * 128],
                            x_tiles[ms][:, cc * 128 : (cc + 1) * 128],
                            ident_f,
                        )
                    nc.scalar.copy(xt[:, cc, :], tp)

                if kind in ("q", "k"):
                    dst = qT if kind == "q" else kT
                    for dc in range(4):
                        ps = ppsum.tile([128, 512], F32, tag="pp")
                        for cc in range(8):
                            nc.tensor.matmul(
                                ps,
                                lhsT=w_sb[:, cc, dc * 128 : (dc + 1) * 128],
                                rhs=xt[:, cc, :],
                                start=(cc == 0),
                                stop=(cc == 7),
                            )
                        nc.scalar.copy(dst[:, dc, mb * 512 : (mb + 1) * 512], ps)
                else:
                    for ms in range(4):
                        ps = ppsum.tile([128, 512], F32, tag="pp")
                        for cc in range(8):
                            nc.tensor.matmul(
                                ps,
                                lhsT=xt[:, cc, ms * 128 : (ms + 1) * 128],
                                rhs=w_sb[:, cc, :],
                                start=(cc == 0),
                                stop=(cc == 7),
                            )
                        nc.scalar.copy(v_bf[:, mb * 4 + ms, :], ps)

    # ---------------- Phase 2: scores, softmax, attention out, PV ----------
    with ExitStack() as ph2:
        spool = ph2.enter_context(tc.tile_pool(name="spool", bufs=2, space="PSUM"))
        tppool = ph2.enter_context(tc.tile_pool(name="tppool", bufs=2, space="PSUM"))
        opool = ph2.enter_context(tc.tile_pool(name="opool", bufs=2, space="PSUM"))
        epool = ph2.enter_context(tc.tile_pool(name="epool", bufs=3))
        pppool = ph2.enter_context(tc.tile_pool(name="pppool", bufs=3))
        ptpool = ph2.enter_context(tc.tile_pool(name="ptpool", bufs=2))
        zpool = ph2.enter_context(tc.tile_pool(name="zpool", bufs=6))

        for hp in range(4):  # head pairs
            for qb in range(4):  # q-blocks of 512
                o_ps = opool.tile([128, 512], F32, tag="ops")
                for par in range(2):  # head within pair
                    h = 2 * hp + par
                    p0, p1 = par * 64, (par + 1) * 64
                    pt = ptpool.tile([128, 16, 512], BF16, tag="pt")
                    for qi in range(4):
                        q0 = qb * 512 + qi * 128
                        e_t = epool.tile([128, S], BF16, tag="e")
                        zc = zpool.tile([128, 2], F32, tag="zc")
                        for kh in range(2):
                            s_ps = spool.tile([128, 1024], F32, tag="s")
                            for ks in range(2):
                                k0 = kh * 1024 + ks * 512
                                nc.tensor.matmul(
                                    s_ps[:, ks * 512 : (ks + 1) * 512],
                                    lhsT=qT[p0:p1, hp, q0 : q0 + 128],
                                    rhs=kT[p0:p1, hp, k0 : k0 + 512],
                                    start=True,
                                    stop=True,
                                )
                            nc.scalar.activation(
                                e_t[:, kh * 1024 : (kh + 1) * 1024],
                                s_ps,
                                mybir.ActivationFunctionType.Exp,
                                scale=0.125,
                                accum_out=zc[:, kh : kh + 1],
                            )
                        zs = zpool.tile([128, 1], F32, tag="zs")
                        nc.vector.tensor_add(zs, zc[:, 0:1], zc[:, 1:2])
                        recip = zpool.tile([128, 1], F32, tag="zr")
                        nc.vector.reciprocal(recip, zs)
                        p_t = pppool.tile([128, S], BF16, tag="p")
                        nc.vector.tensor_scalar_mul(p_t, e_t, recip)
                        nc.gpsimd.dma_start(out=att[h, q0 : q0 + 128, :], in_=p_t)
                        for kc in range(16):
                            tp = tppool.tile([128, 128], BF16, tag="tpp")
                            nc.tensor.transpose(
                                tp,
                                p_t[:, kc * 128 : (kc + 1) * 128],
                                ident_b,
                            )
                            nc.vector.tensor_copy(
                                pt[:, kc, qi * 128 : (qi + 1) * 128], tp
                            )
                    for kc in range(16):
                        nc.tensor.matmul(
                            o_ps[p0:p1, :],
                            lhsT=v_bf[:, kc, h * 64 : (h + 1) * 64],
                            rhs=pt[:, kc, :],
                            start=(kc == 0),
                            stop=(kc == 15),
                        )
                nc.scalar.copy(oT[:, hp, qb * 512 : (qb + 1) * 512], o_ps)

    # ---------------- Phase 3: output projection ---------------------------
    with ExitStack() as ph3:
        fpool = ph3.enter_context(tc.tile_pool(name="fpool", bufs=2, space="PSUM"))
        outpool = ph3.enter_context(tc.tile_pool(name="outpool", bufs=2))
        for qk in range(16):
            ps_a = fpool.tile([128, 512], F32, tag="fa")
            ps_b = fpool.tile([128, 512], F32, tag="fb")
            for pc in range(4):
                lhs = oT[:, pc, qk * 128 : (qk + 1) * 128]
                nc.tensor.matmul(
                    ps_a, lhsT=lhs, rhs=wo_bf[:, pc, 0:512],
                    start=(pc == 0), stop=(pc == 3),
                )
                nc.tensor.matmul(
                    ps_b, lhsT=lhs, rhs=wo_bf[:, pc, 512:1024],
                    start=(pc == 0), stop=(pc == 3),
                )
            osb = outpool.tile([128, D], F32, tag="osb")
            nc.vector.tensor_copy(osb[:, 0:512], ps_a)
            nc.vector.tensor_copy(osb[:, 512:1024], ps_b)
            nc.sync.dma_start(out=outp[qk * 128 : (qk + 1) * 128, :], in_=osb)


_PROGRAM = None


def _get_program():
    global _PROGRAM
    if _PROGRAM is None:
        _PROGRAM = build_program()
    return _PROGRAM


def _numpy_reference(query, key_, value, mask, Wq, bq, Wk, bk, Wv, bv, Wo, bo):
    def proj(x, W, b):
        return (x @ W + b).reshape(B, -1, H, HD).transpose(0, 2, 1, 3)

    q = proj(query, Wq, bq)
    k = proj(key_, Wk, bk)
    v = proj(value, Wv, bv)
    att = np.einsum("bhqd,bhkd->bhqk", q, k) / np.sqrt(np.float32(HD))
    att = np.where(mask == 0, -np.inf, att)
    att = att - att.max(axis=-1, keepdims=True)
    e = np.exp(att)
    p = e / e.sum(axis=-1, keepdims=True)
    out = np.einsum("bhqk,bhkd->bhqd", p, v)
    out = out.transpose(0, 2, 1, 3).reshape(B, -1, D)
    out = out @ Wo + bo
    return out.astype(np.float32), p.astype(np.float32)


def kernel(query, key_, value, mask, Wq, bq, Wk, bk, Wv, bv, Wo, bo):
    query = np.asarray(query, dtype=np.float32)
    key_ = np.asarray(key_, dtype=np.float32)
    value = np.asarray(value, dtype=np.float32)
    mask = np.asarray(mask)
    Wq = np.asarray(Wq, dtype=np.float32)
    Wk = np.asarray(Wk, dtype=np.float32)
    Wv = np.asarray(Wv, dtype=np.float32)
    Wo = np.asarray(Wo, dtype=np.float32)
    bq = np.asarray(bq, dtype=np.float32)
    bk = np.asarray(bk, dtype=np.float32)
    bv = np.asarray(bv, dtype=np.float32)
    bo = np.asarray(bo, dtype=np.float32)

    if (
        not np.all(mask != 0)
        or np.any(bq != 0)
        or np.any(bk != 0)
        or np.any(bv != 0)
    ):
        # Masked softmax / nonzero-bias fast paths not built; fall back.
        return _numpy_reference(
            query, key_, value, mask, Wq, bq, Wk, bk, Wv, bv, Wo, bo
        )

    nc = _get_program()
    in_maps = []
    for core in range(NCORES):
        b, g = core // 2, core % 2
        sl = slice(g * DG, (g + 1) * DG)
        in_maps.append(
            {
                "xq": query[b],
                "xk": key_[b],
                "xv": value[b],
                "wq": np.ascontiguousarray(Wq[:, sl]),
                "wk": np.ascontiguousarray(Wk[:, sl]),
                "wv": np.ascontiguousarray(Wv[:, sl]),
                "wo": np.ascontiguousarray(Wo[sl, :]),
            }
        )

    res = run_bass_kernel_spmd(nc, in_maps, core_ids=list(range(NCORES)))

    attention = np.empty((B, H, S, S), dtype=np.float32)
    out = np.empty((B, S, D), dtype=np.float32)
    for b in range(B):
        attention[b, 0:HG] = res.results[2 * b]["att"]
        attention[b, HG:H] = res.results[2 * b + 1]["att"]
        out[b] = res.results[2 * b]["outp"] + res.results[2 * b + 1]["outp"] + bo
    return out, attention
